# revision 1
# baseline (speedup 1.0000x reference)
"""Trainium2 Bass kernel for nn_Conv2dGeneral (capsule-style 4x4-pose conv).

Math (from the reference):
  out[b,o,X,Y,u,w] = sum_{cin,kx,ky,v} Wm[(cin,kx,ky),o,u,v] * x[b,cin,2X+kx,2Y+ky,4v+w] + bias[o]

Mapped to the PE array as a single 1152-deep contraction:
  K = (cin, v)  x  9 accumulation chunks over (kx, ky)   [9 x 128]
  M = (o, u)                                              [128 PSUM partitions]
  N = (X, Y, w)                                           [676 per batch image]

Data-parallel across 8 NeuronCores on the batch dim (8 images per core).

Host-side prep: x is re-laid-out to [(b), (cin,v), (r,c,w)] so each core's
shard DMAs as fully-contiguous 12.5KB partition lines; the stride-2 im2col
window gather then happens for free inside the matmul moving-operand access
pattern (no patch materialization, each x byte read once from HBM).
"""

import numpy as np

B, CIN, COUT = 64, 32, 32
KK, STRIDE = 3, 2
WIN, HH = 28, 16
H = 4
WOUT = (WIN - KK) // STRIDE + 1  # 13
NCORES = 8
BPC = B // NCORES                # batches per core
RCW = WIN * WIN * H              # 3136 free elements per (cin,v) partition
NOUT = WOUT * WOUT * H           # 676 outputs per (o,u) partition per image
XSPLIT = ((0, 7), (7, 6))        # two PSUM groups: X rows [0,7) and [7,13)

_cache = {}


def _build_bass():
    """Raw-bass build (no Tile): this toolchain's walrus codegen allows only
    ONE sync-wait per instruction, so all cross-engine sync is explicit
    single-sem waits; ordering beyond that rides on hardware transitivity.

    Engines: SP triggers the 7 DMAs, PE runs 16 accumulation groups of 9
    matmuls (one per kernel tap), ACT evicts PSUM->SBUF adding the bias.
    """
    import concourse.bass as bass
    import concourse.mybir as mybir

    f32 = mybir.dt.float32
    f16 = mybir.dt.float16
    OB = 2                    # batches per output-DMA chunk
    NOC = BPC // OB           # 4 output chunks
    NG = 2 * BPC              # 16 PSUM accumulation groups
    GPO = 2 * OB              # groups per output chunk
    WARMUP = 24               # PE warm-up matmuls while x[0] streams in

    nc = bass.Bass()
    x_d = nc.declare_dram_parameter("x", [BPC, 128, RCW], f16, isOutput=False)
    w_d = nc.declare_dram_parameter("w", [128, 9 * 128], f16, isOutput=False)
    b_d = nc.declare_dram_parameter("b", [128, 1], f32, isOutput=False)
    o_d = nc.declare_dram_parameter("out", [NOC, 128, OB * NOUT], f32, isOutput=True)

    with (
        nc.sbuf_tensor([128, 9 * 128], f16) as wt,
        nc.sbuf_tensor([128, 1], f32) as bt,
        nc.sbuf_tensor([128, BPC, RCW], f16) as gt,
        nc.sbuf_tensor([128, NOC, OB * NOUT], f32) as ot,
        nc.psum_tensor([128, 8, 512], f32) as ps,
        nc.semaphore("wt_sem") as wt_sem,
        nc.semaphore("bias_sem") as bias_sem,
        nc.semaphore("g_sem0") as g_sem0,
        nc.semaphore("g_sem1") as g_sem1,
        nc.semaphore("g_sem2") as g_sem2,
        nc.semaphore("g_sem3") as g_sem3,
        nc.semaphore("g_sem4") as g_sem4,
        nc.semaphore("g_sem5") as g_sem5,
        nc.semaphore("g_sem6") as g_sem6,
        nc.semaphore("g_sem7") as g_sem7,
        nc.semaphore("pe_sem") as pe_sem,
        nc.semaphore("act_sem") as act_sem,
        nc.semaphore("out_sem") as out_sem,
        nc.Block() as block,
    ):
        g_sems = [g_sem0, g_sem1, g_sem2, g_sem3, g_sem4, g_sem5, g_sem6, g_sem7]
        wtr = wt[:, :].rearrange("p (k m) -> p k m", k=9)

        @block.sync
        def _(sync):
            sync.dma_start(wt[:, :], w_d[:, :]).then_inc(wt_sem, 16)
            sync.dma_start(bt[:, :], b_d[:, :]).then_inc(bias_sem, 16)
            for b in range(BPC):
                sync.dma_start(gt[:, b, :], x_d[b]).then_inc(g_sems[b], 16)
            sync.wait_ge(out_sem, 16 * NOC)

        @block.tensor
        def _(tensor):
            tensor.wait_ge(wt_sem, 16)
            # Warm the PE HAM clock gate (cold = 1.2 GHz) while x streams in.
            for i in range(WARMUP):
                tensor.matmul(
                    ps[:, 7, :128], wt[:, :128], wt[:, :128], start=True, stop=True
                )
            for j in range(NG):
                b, half = divmod(j, 2)
                if half == 0:
                    tensor.wait_ge(g_sems[b], 16)
                if j >= 8:
                    # PSUM bank j%8 is free once ACT drained group j-8
                    tensor.wait_ge(act_sem, j - 7)
                X0, nX = XSPLIT[half]
                gr = gt[:, b, :].rearrange("p (r c w) -> p r c w", r=WIN, c=WIN)
                for kk in range(9):
                    kx, ky = divmod(kk, 3)
                    rhs = gr[
                        :,
                        2 * X0 + kx : 2 * X0 + kx + 2 * nX - 1 : 2,
                        ky : ky + 2 * WOUT - 1 : 2,
                        :,
                    ]
                    mm = tensor.matmul(
                        ps[:, j % 8, : nX * WOUT * H],
                        wtr[:, kk, :],
                        rhs,
                        start=(kk == 0),
                        stop=(kk == 8),
                    )
                mm.then_inc(pe_sem, 1)

        @block.scalar
        def _(scalar):
            scalar.wait_ge(bias_sem, 16)
            for j in range(NG):
                b, half = divmod(j, 2)
                X0, nX = XSPLIT[half]
                oc, obi = divmod(b, OB)
                off = obi * NOUT + X0 * WOUT * H
                scalar.wait_ge(pe_sem, j + 1)
                scalar.activation(
                    ot[:, oc, off : off + nX * WOUT * H],
                    ps[:, j % 8, : nX * WOUT * H],
                    mybir.ActivationFunctionType.Identity,
                    bias=bt[:, :],
                ).then_inc(act_sem, 1)
                if j % GPO == GPO - 1:
                    # output chunk complete; ship it from the ACT ring
                    scalar.dma_start(o_d[j // GPO], ot[:, j // GPO, :]).then_inc(
                        out_sem, 16
                    )

    return nc


def _prep_inputs(x, W, bias):
    # x: (B, CIN, 28, 28, 16) -> xp[b, cin*4+v, (r*28+c)*4+w] = x[b,cin,r,c,4v+w]
    # fp16: PE runs fp32 matmuls as LOW_HIGH double passes; fp16 is single-pass
    # with fast-weight-load, and halves the dominant HBM traffic. Max rel err
    # ~3e-4 at this contraction depth (fp32 PSUM accumulation).
    xp = np.ascontiguousarray(
        x.reshape(B, CIN, WIN, WIN, H, H).transpose(0, 1, 4, 2, 3, 5)
    ).reshape(B, CIN * H, RCW).astype(np.float16)
    # W: (1, 288, 32, 1, 1, 4, 4); p = cin*9 + kx*3 + ky
    # wt_sb[cin*4+v, kk*128 + o*4+u] = Wm[cin*9+kk, o, u, v]
    Wm = np.asarray(W, dtype=np.float32).reshape(CIN, KK * KK, COUT, H, H)
    wt_sb = np.ascontiguousarray(
        Wm.transpose(0, 4, 1, 2, 3)  # cin, v, kk, o, u
    ).reshape(128, 9 * 128).astype(np.float16)
    bias_v = np.ascontiguousarray(
        np.repeat(np.asarray(bias, dtype=np.float32).reshape(COUT), H)
    ).reshape(128, 1)
    return xp, wt_sb, bias_v


def _shard_x(xp, core):
    # per-core input: [BPC, 128, RCW] fp16
    return np.ascontiguousarray(xp[core * BPC : (core + 1) * BPC])


def _unchunk_out(dev_out, ob=2):
    # dev_out: (BPC//ob, 128, ob*NOUT) -> (BPC, 128, NOUT)
    return (
        dev_out.reshape(BPC // ob, 128, ob, NOUT)
        .transpose(0, 2, 1, 3)
        .reshape(BPC, 128, NOUT)
    )


def _unprep_output(full):
    # full: (B, 128, NOUT) with partition o*4+u, free (X, Y, w)
    out = (
        full.reshape(B, COUT, H, WOUT, WOUT, H)
        .transpose(0, 1, 3, 4, 2, 5)
        .reshape(B, COUT, WOUT, WOUT, HH)
    )
    return np.ascontiguousarray(out)


def run_device(in_maps, trace=False, tmpdir=None):
    from concourse.bass_utils import run_bass_kernel_spmd

    if "nc" not in _cache:
        _cache["nc"] = _build_bass()
    return run_bass_kernel_spmd(
        _cache["nc"], in_maps, list(range(NCORES)), trace=trace, tmpdir=tmpdir
    )


def kernel(x, W, bias):
    x = np.asarray(x, dtype=np.float32)
    xp, wt_sb, bias_v = _prep_inputs(x, W, bias)
    in_maps = [
        {"x": _shard_x(xp, i), "w": wt_sb, "b": bias_v} for i in range(NCORES)
    ]
    res = run_device(in_maps, trace=False)
    full = np.concatenate(
        [_unchunk_out(res.results[i]["out"]) for i in range(NCORES)], axis=0
    )
    return _unprep_output(full)



# revision 7
# speedup vs baseline: 1.0233x; 1.0233x over previous
"""Trainium2 Bass kernel for nn_Conv2dGeneral (capsule-style 4x4-pose conv).

Math (from the reference):
  out[b,o,X,Y,u,w] = sum_{cin,kx,ky,v} Wm[(cin,kx,ky),o,u,v] * x[b,cin,2X+kx,2Y+ky,4v+w] + bias[o]

Mapped to the PE array as a single 1152-deep contraction:
  K = (cin, v)  x  9 accumulation chunks over (kx, ky)   [9 x 128]
  M = (o, u)                                              [128 PSUM partitions]
  N = (X, Y, w)                                           [676 per batch image]

Data-parallel across 8 NeuronCores on the batch dim (8 images per core).

Timeline engineering (vs the first working version):
  - PE warm-up matmuls read an uninitialized SBUF tile, so they need no
    semaphore wait and ramp the HAM clock during the fixed NEFF preamble.
  - x streams through BOTH hardware DGE queues (sync + scalar): each image
    is split into head rows [0,15) and tail rows [14,27) (row 27 is never
    read by the stride-2 K=3 window), alternating queues per batch. One
    semaphore per queue; FIFO position encodes which transfer completed.
  - Output is written as fp16 (the fp32 reference tolerates it at ~5e-4
    rel err) in 16 per-half-image chunks, each DMA'd right after its
    PSUM-evicting activation, keeping the post-compute tail short.
  - A dummy activation at scalar program start hoists the 1.3us
    ACT_TABLE_LOAD into the startup shadow.
"""

import numpy as np

B, CIN, COUT = 64, 32, 32
KK, STRIDE = 3, 2
WIN, HH = 28, 16
H = 4
WOUT = (WIN - KK) // STRIDE + 1  # 13
NCORES = 8
BPC = B // NCORES                # batches per core
RCW = WIN * WIN * H              # 3136 free elements per (cin,v) partition
NOUT = WOUT * WOUT * H           # 676 outputs per (o,u) partition per image
XSPLIT = ((0, 7), (7, 6))        # two PSUM groups: X rows [0,7) and [7,13)
ROWELE = WIN * H                 # 112 elements per image row per partition
HEAD_ROWS = (0, 15)              # covers X in [0,7): rows 2X+kx <= 14
TAIL_ROWS = (14, 27)             # covers X in [7,13): rows 14..26
WARMUP = 24                      # PE clock-ramp matmuls (garbage operands)

_cache = {}


def _build_bass():
    """Raw-bass build (no Tile): this toolchain's walrus codegen allows only
    ONE sync-wait per instruction, so all cross-engine sync is explicit
    single-sem waits; ordering beyond that rides on hardware transitivity.
    """
    import concourse.bass as bass
    import concourse.mybir as mybir

    f32 = mybir.dt.float32
    f16 = mybir.dt.float16
    NG = 2 * BPC              # 16 PSUM accumulation groups (half-images)

    nc = bass.Bass()
    x_d = nc.declare_dram_parameter("x", [BPC, 128, RCW], f16, isOutput=False)
    w_d = nc.declare_dram_parameter("w", [128, 9 * 128], f16, isOutput=False)
    b_d = nc.declare_dram_parameter("b", [128, 1], f32, isOutput=False)
    o_d = nc.declare_dram_parameter("out", [BPC, 128, NOUT], f16, isOutput=True)

    h0, h1 = HEAD_ROWS[0] * ROWELE, HEAD_ROWS[1] * ROWELE
    t0, t1 = TAIL_ROWS[0] * ROWELE, TAIL_ROWS[1] * ROWELE

    from contextlib import ExitStack

    with ExitStack() as es:
        wt = es.enter_context(nc.sbuf_tensor([128, 9 * 128], f16))
        bt = es.enter_context(nc.sbuf_tensor([128, 1], f32))
        gt = es.enter_context(nc.sbuf_tensor([128, BPC, RCW], f16))
        ot = es.enter_context(nc.sbuf_tensor([128, BPC * NOUT], f16))
        junk = es.enter_context(nc.sbuf_tensor([128, 128], f16))
        junk_out = es.enter_context(nc.sbuf_tensor([128, 16], f16))
        ps = es.enter_context(nc.psum_tensor([128, 8, 512], f32))
        wt_sem = es.enter_context(nc.semaphore("wt_sem"))
        bias_sem = es.enter_context(nc.semaphore("bias_sem"))
        h_sems = [es.enter_context(nc.semaphore(f"h{b}s")) for b in range(BPC)]
        t_sems = [es.enter_context(nc.semaphore(f"t{b}s")) for b in range(BPC)]
        pe_sem = es.enter_context(nc.semaphore("pe_sem"))
        act_sem = es.enter_context(nc.semaphore("act_sem"))
        out_sem = es.enter_context(nc.semaphore("out_sem"))
        block = es.enter_context(nc.Block())
        wtr = wt[:, :].rearrange("p (k m) -> p k m", k=9)

        # Two HW DGE queues stream x concurrently, alternating head/tail per
        # batch: Q1(sync) = [w, bias, h0, t1, h2, t3, ...],
        # Q10(scalar) = [t0, h1, t2, h3, ...]. Per-transfer semaphores (+16
        # at completion = +1 per DMA engine slice).
        @block.sync
        def _(sync):
            sync.dma_start(wt[:, :], w_d[:, :]).then_inc(wt_sem, 16)
            sync.dma_start(bt[:, :], b_d[:, :]).then_inc(bias_sem, 16)
            for b in range(BPC):
                if b % 2 == 0:
                    sync.dma_start(
                        gt[:, b, h0:h1], x_d[b][:, h0:h1]
                    ).then_inc(h_sems[b], 16)
                else:
                    sync.dma_start(
                        gt[:, b, t0:t1], x_d[b][:, t0:t1]
                    ).then_inc(t_sems[b], 16)
            sync.wait_ge(out_sem, 16 * NG)

        @block.tensor
        def _(tensor):
            # Warm the PE HAM clock gate (cold = 1.2 GHz) on garbage SBUF —
            # no semaphore wait, so this runs in the DMA-latency shadow.
            for i in range(WARMUP):
                tensor.matmul(
                    ps[:, 7, :128], junk[:, :], junk[:, :], start=True, stop=True
                )
            tensor.wait_ge(wt_sem, 16)
            for j in range(NG):
                b, half = divmod(j, 2)
                if half == 0:
                    tensor.wait_ge(h_sems[b], 16)
                else:
                    tensor.wait_ge(t_sems[b], 16)
                if j >= 8:
                    # PSUM bank j%8 is free once ACT drained group j-8
                    tensor.wait_ge(act_sem, j - 7)
                X0, nX = XSPLIT[half]
                gr = gt[:, b, :].rearrange("p (r c w) -> p r c w", r=WIN, c=WIN)
                for kk in range(9):
                    kx, ky = divmod(kk, 3)
                    rhs = gr[
                        :,
                        2 * X0 + kx : 2 * X0 + kx + 2 * nX - 1 : 2,
                        ky : ky + 2 * WOUT - 1 : 2,
                        :,
                    ]
                    mm = tensor.matmul(
                        ps[:, j % 8, : nX * WOUT * H],
                        wtr[:, kk, :],
                        rhs,
                        start=(kk == 0),
                        stop=(kk == 8),
                    )
                mm.then_inc(pe_sem, 1)

        @block.scalar
        def _(scalar):
            for b in range(BPC):
                if b % 2 == 0:
                    scalar.dma_start(
                        gt[:, b, t0:t1], x_d[b][:, t0:t1]
                    ).then_inc(t_sems[b], 16)
                else:
                    scalar.dma_start(
                        gt[:, b, h0:h1], x_d[b][:, h0:h1]
                    ).then_inc(h_sems[b], 16)
            # Hoist ACT_TABLE_LOAD: walrus emits it before the first ACT.
            # Reads garbage (PSUM bank 7 mid-warmup + unloaded bt) — discarded.
            scalar.activation(
                junk_out[:, :],
                ps[:, 7, :16],
                mybir.ActivationFunctionType.Identity,
                bias=bt[:, :],
            )
            scalar.wait_ge(bias_sem, 16)
            for j in range(NG):
                b, half = divmod(j, 2)
                X0, nX = XSPLIT[half]
                off = b * NOUT + X0 * WOUT * H
                n = nX * WOUT * H
                scalar.wait_ge(pe_sem, j + 1)
                scalar.activation(
                    ot[:, off : off + n],
                    ps[:, j % 8, :n],
                    mybir.ActivationFunctionType.Identity,
                    bias=bt[:, :],
                ).then_inc(act_sem, 1)
                doff = X0 * WOUT * H
                scalar.dma_start(
                    o_d[b][:, doff : doff + n], ot[:, off : off + n]
                ).then_inc(out_sem, 16)

    return nc


def _prep_inputs(x, W, bias):
    # x: (B, CIN, 28, 28, 16) -> xp[b, cin*4+v, (r*28+c)*4+w] = x[b,cin,r,c,4v+w]
    # fp16: PE runs fp32 matmuls as LOW_HIGH double passes; fp16 is single-pass
    # with fast-weight-load, and halves the dominant HBM traffic. Max rel err
    # ~3e-4 at this contraction depth (fp32 PSUM accumulation).
    xp = np.ascontiguousarray(
        x.reshape(B, CIN, WIN, WIN, H, H).transpose(0, 1, 4, 2, 3, 5)
    ).reshape(B, CIN * H, RCW).astype(np.float16)
    # W: (1, 288, 32, 1, 1, 4, 4); p = cin*9 + kx*3 + ky
    # wt_sb[cin*4+v, kk*128 + o*4+u] = Wm[cin*9+kk, o, u, v]
    Wm = np.asarray(W, dtype=np.float32).reshape(CIN, KK * KK, COUT, H, H)
    wt_sb = np.ascontiguousarray(
        Wm.transpose(0, 4, 1, 2, 3)  # cin, v, kk, o, u
    ).reshape(128, 9 * 128).astype(np.float16)
    bias_v = np.ascontiguousarray(
        np.repeat(np.asarray(bias, dtype=np.float32).reshape(COUT), H)
    ).reshape(128, 1)
    return xp, wt_sb, bias_v


def _shard_x(xp, core):
    # per-core input: [BPC, 128, RCW] fp16
    return np.ascontiguousarray(xp[core * BPC : (core + 1) * BPC])


def _unprep_output(full):
    # full: (B, 128, NOUT) fp16 with partition o*4+u, free (X, Y, w)
    out = (
        full.astype(np.float32)
        .reshape(B, COUT, H, WOUT, WOUT, H)
        .transpose(0, 1, 3, 4, 2, 5)
        .reshape(B, COUT, WOUT, WOUT, HH)
    )
    return np.ascontiguousarray(out)


def run_device(in_maps, trace=False, tmpdir=None):
    from concourse.bass_utils import run_bass_kernel_spmd

    if "nc" not in _cache:
        _cache["nc"] = _build_bass()
    return run_bass_kernel_spmd(
        _cache["nc"], in_maps, list(range(NCORES)), trace=trace, tmpdir=tmpdir
    )


def kernel(x, W, bias):
    x = np.asarray(x, dtype=np.float32)
    xp, wt_sb, bias_v = _prep_inputs(x, W, bias)
    in_maps = [
        {"x": _shard_x(xp, i), "w": wt_sb, "b": bias_v} for i in range(NCORES)
    ]
    res = run_device(in_maps, trace=False)
    full = np.concatenate(
        [res.results[i]["out"] for i in range(NCORES)], axis=0
    )
    return _unprep_output(full)


# revision 16
# speedup vs baseline: 1.1057x; 1.0805x over previous
"""Trainium2 Bass kernel for nn_Conv2dGeneral (capsule-style 4x4-pose conv).

Math (from the reference):
  out[b,o,X,Y,u,w] = sum_{cin,kx,ky,v} Wm[(cin,kx,ky),o,u,v] * x[b,cin,2X+kx,2Y+ky,4v+w] + bias[o]

Mapped to the PE array as a single 1152-deep contraction:
  K = (cin, v)  x  9 accumulation chunks over (kx, ky)   [9 x 128]
  M = (o, u)                                              [128 PSUM partitions]
  N = (X, Y, w)                                           [676 per batch image]

Data-parallel across 8 NeuronCores on the batch dim (8 images per core).

Timeline engineering (vs the first working version):
  - PE warm-up matmuls read an uninitialized SBUF tile, so they need no
    semaphore wait and ramp the HAM clock during the fixed NEFF preamble.
  - x streams through BOTH hardware DGE queues (sync + scalar): each image
    is split into head rows [0,15) and tail rows [14,27) (row 27 is never
    read by the stride-2 K=3 window), alternating queues per batch. One
    semaphore per queue; FIFO position encodes which transfer completed.
  - Output is written as fp16 (the fp32 reference tolerates it at ~5e-4
    rel err) in 16 per-half-image chunks, each DMA'd right after its
    PSUM-evicting activation, keeping the post-compute tail short.
  - A dummy activation at scalar program start hoists the 1.3us
    ACT_TABLE_LOAD into the startup shadow.
"""

import numpy as np

B, CIN, COUT = 64, 32, 32
KK, STRIDE = 3, 2
WIN, HH = 28, 16
H = 4
WOUT = (WIN - KK) // STRIDE + 1  # 13
NCORES = 8
BPC = B // NCORES                # batches per core
RCW = WIN * WIN * H              # 3136 free elements per (cin,v) partition
NOUT = WOUT * WOUT * H           # 676 outputs per (o,u) partition per image
XSPLIT = ((0, 7), (7, 6))        # two PSUM groups: X rows [0,7) and [7,13)
ROWELE = WIN * H                 # 112 elements per image row per partition
HEAD_ROWS = (0, 15)              # covers X in [0,7): rows 2X+kx <= 14
TAIL_ROWS = (15, 27)             # with head's row 14: covers X in [7,13)
WARMUP = 24                      # PE clock-ramp matmuls (garbage operands)

_cache = {}


def _build_bass():
    """Raw-bass build (no Tile): this toolchain's walrus codegen allows only
    ONE sync-wait per instruction, so all cross-engine sync is explicit
    single-sem waits; ordering beyond that rides on hardware transitivity.
    """
    import concourse.bass as bass
    import concourse.mybir as mybir

    f32 = mybir.dt.float32
    f16 = mybir.dt.float16
    NG = 2 * BPC              # 16 PSUM accumulation groups (half-images)

    nc = bass.Bass()
    x_d = nc.declare_dram_parameter("x", [BPC, 128, RCW], f16, isOutput=False)
    w_d = nc.declare_dram_parameter("w", [128, 9 * 128], f16, isOutput=False)
    b_d = nc.declare_dram_parameter("b", [128, 1], f32, isOutput=False)
    o_d = nc.declare_dram_parameter("out", [BPC, 128, NOUT], f16, isOutput=True)

    HE1 = HEAD_ROWS[1] * ROWELE   # head = rows [0,15)
    TE1 = TAIL_ROWS[1] * ROWELE   # tail = rows [15,27); row 27 never read

    from contextlib import ExitStack

    with ExitStack() as es:
        wt = es.enter_context(nc.sbuf_tensor([128, 9 * 128], f16))
        bt = es.enter_context(nc.sbuf_tensor([128, 1], f32))
        gt = es.enter_context(nc.sbuf_tensor([128, BPC, RCW], f16))
        ot = es.enter_context(nc.sbuf_tensor([128, BPC * NOUT], f16))
        junk = es.enter_context(nc.sbuf_tensor([128, 128], f16))
        junk_out = es.enter_context(nc.sbuf_tensor([128, 16], f16))
        ps = es.enter_context(nc.psum_tensor([128, 8, 512], f32))
        wt_sem = es.enter_context(nc.semaphore("wt_sem"))
        bias_sem = es.enter_context(nc.semaphore("bias_sem"))
        h0_sem = es.enter_context(nc.semaphore("h0s"))
        t0_sem = es.enter_context(nc.semaphore("t0s"))
        g_sems = [es.enter_context(nc.semaphore(f"g{b}s")) for b in range(1, BPC)]
        pe_sem = es.enter_context(nc.semaphore("pe_sem"))
        act_sem = es.enter_context(nc.semaphore("act_sem"))
        out_sem = es.enter_context(nc.semaphore("out_sem"))
        junk_sem = es.enter_context(nc.semaphore("junk_sem"))
        block = es.enter_context(nc.Block())
        wtr = wt[:, :].rearrange("p (k m) -> p k m", k=9)

        # All x rides the sync HW queue in priority order — the 16 DMA
        # engines fair-share between queues with backlog, so a second
        # backlogged input queue just halves this one's rate. Batch 0 is
        # row-split so its first-half matmuls start ~1.3us sooner; row 27
        # is never read by the stride-2 K=3 window and isn't transferred.
        @block.sync
        def _(sync):
            sync.dma_start(wt[:, :], w_d[:, :]).then_inc(wt_sem, 16)
            sync.dma_start(gt[:, 0, :HE1], x_d[0][:, :HE1]).then_inc(h0_sem, 16)
            sync.dma_start(
                gt[:, 0, HE1:TE1], x_d[0][:, HE1:TE1]
            ).then_inc(t0_sem, 16)
            for b in range(1, BPC):
                sync.dma_start(
                    gt[:, b, :TE1], x_d[b][:, :TE1]
                ).then_inc(g_sems[b - 1], 16)
            sync.wait_ge(out_sem, 16 * NG)

        @block.gpsimd
        def _(gpsimd):
            gpsimd.memset(junk[:, :], 0.0).then_inc(junk_sem, 1)

        @block.tensor
        def _(tensor):
            # Warm the PE HAM clock gate (cold = 1.2 GHz) on zeroed SBUF —
            # no DMA wait, so this runs in the DMA-latency shadow.
            tensor.wait_ge(junk_sem, 1)
            for i in range(WARMUP):
                tensor.matmul(
                    ps[:, 7, :128], junk[:, :], junk[:, :], start=True, stop=True
                )
            tensor.wait_ge(wt_sem, 16)
            for j in range(NG):
                b, half = divmod(j, 2)
                # half1 of each batch needs no extra wait: half0 already
                # waited on this batch's data earlier in program order.
                if j == 0:
                    tensor.wait_ge(h0_sem, 16)
                elif j == 1:
                    tensor.wait_ge(t0_sem, 16)
                elif half == 0:
                    tensor.wait_ge(g_sems[b - 1], 16)
                if j >= 8:
                    # PSUM bank j%8 is free once ACT drained group j-8
                    tensor.wait_ge(act_sem, j - 7)
                X0, nX = XSPLIT[half]
                gr = gt[:, b, :].rearrange("p (r c w) -> p r c w", r=WIN, c=WIN)
                for kk in range(9):
                    kx, ky = divmod(kk, 3)
                    rhs = gr[
                        :,
                        2 * X0 + kx : 2 * X0 + kx + 2 * nX - 1 : 2,
                        ky : ky + 2 * WOUT - 1 : 2,
                        :,
                    ]
                    mm = tensor.matmul(
                        ps[:, j % 8, : nX * WOUT * H],
                        wtr[:, kk, :],
                        rhs,
                        start=(kk == 0),
                        stop=(kk == 8),
                    )
                mm.then_inc(pe_sem, 1)

        @block.scalar
        def _(scalar):
            # bias rides the otherwise-idle scalar HW queue, off Q1's path
            scalar.dma_start(bt[:, :], b_d[:, :]).then_inc(bias_sem, 16)
            # Hoist ACT_TABLE_LOAD: walrus emits it before the first ACT.
            # Reads zeroed junk with a const bias — no data dependency.
            scalar.wait_ge(junk_sem, 1)
            scalar.activation(
                junk_out[:, :],
                junk[:, :16],
                mybir.ActivationFunctionType.Identity,
                bias=0.0,
            )
            scalar.wait_ge(bias_sem, 16)
            for j in range(NG):
                b, half = divmod(j, 2)
                X0, nX = XSPLIT[half]
                off = b * NOUT + X0 * WOUT * H
                n = nX * WOUT * H
                scalar.wait_ge(pe_sem, j + 1)
                scalar.activation(
                    ot[:, off : off + n],
                    ps[:, j % 8, :n],
                    mybir.ActivationFunctionType.Identity,
                    bias=bt[:, :],
                ).then_inc(act_sem, 1)
                # self-wait: ACT write-back posted before the DMA doorbell
                scalar.wait_ge(act_sem, j + 1)
                doff = X0 * WOUT * H
                scalar.dma_start(
                    o_d[b][:, doff : doff + n], ot[:, off : off + n]
                ).then_inc(out_sem, 16)

    return nc


def _prep_inputs(x, W, bias):
    # x: (B, CIN, 28, 28, 16) -> xp[b, cin*4+v, (r*28+c)*4+w] = x[b,cin,r,c,4v+w]
    # fp16: PE runs fp32 matmuls as LOW_HIGH double passes; fp16 is single-pass
    # with fast-weight-load, and halves the dominant HBM traffic. Max rel err
    # ~3e-4 at this contraction depth (fp32 PSUM accumulation).
    xp = np.ascontiguousarray(
        x.reshape(B, CIN, WIN, WIN, H, H).transpose(0, 1, 4, 2, 3, 5)
    ).reshape(B, CIN * H, RCW).astype(np.float16)
    # W: (1, 288, 32, 1, 1, 4, 4); p = cin*9 + kx*3 + ky
    # wt_sb[cin*4+v, kk*128 + o*4+u] = Wm[cin*9+kk, o, u, v]
    Wm = np.asarray(W, dtype=np.float32).reshape(CIN, KK * KK, COUT, H, H)
    wt_sb = np.ascontiguousarray(
        Wm.transpose(0, 4, 1, 2, 3)  # cin, v, kk, o, u
    ).reshape(128, 9 * 128).astype(np.float16)
    bias_v = np.ascontiguousarray(
        np.repeat(np.asarray(bias, dtype=np.float32).reshape(COUT), H)
    ).reshape(128, 1)
    return xp, wt_sb, bias_v


def _shard_x(xp, core):
    # per-core input: [BPC, 128, RCW] fp16
    return np.ascontiguousarray(xp[core * BPC : (core + 1) * BPC])


def _unprep_output(full):
    # full: (B, 128, NOUT) fp16 with partition o*4+u, free (X, Y, w)
    out = (
        full.astype(np.float32)
        .reshape(B, COUT, H, WOUT, WOUT, H)
        .transpose(0, 1, 3, 4, 2, 5)
        .reshape(B, COUT, WOUT, WOUT, HH)
    )
    return np.ascontiguousarray(out)


def run_device(in_maps, trace=False, tmpdir=None):
    from concourse.bass_utils import run_bass_kernel_spmd

    if "nc" not in _cache:
        _cache["nc"] = _build_bass()
    return run_bass_kernel_spmd(
        _cache["nc"], in_maps, list(range(NCORES)), trace=trace, tmpdir=tmpdir
    )


def kernel(x, W, bias):
    x = np.asarray(x, dtype=np.float32)
    xp, wt_sb, bias_v = _prep_inputs(x, W, bias)
    in_maps = [
        {"x": _shard_x(xp, i), "w": wt_sb, "b": bias_v} for i in range(NCORES)
    ]
    res = run_device(in_maps, trace=False)
    full = np.concatenate(
        [res.results[i]["out"] for i in range(NCORES)], axis=0
    )
    return _unprep_output(full)
